# revision 1
# baseline (speedup 1.0000x reference)
"""Multihead attention (B=2, L=2048, D=1024, 16 heads) on 8 trn2 cores.

Sharding: tensor-parallel over heads — 2 heads per core. Each core computes
q/k/v projections for its 128 columns of Wq/Wk/Wv, full attention for its two
heads, and a partial output projection against its 128 rows of Wo. The host
sums the 8 bf16 partials and adds bo.

Compute is bf16 on the PE with fp32 PSUM accumulation. Per-core layouts (all
built from a host-side transpose+cast of x):
  qT/kT: [128(d_local), B*L]   — contraction-major for the scoresT matmuls
  v:     [s, 64]+ones column   — lhsT for attn@v; the ones column makes the
                                 PE emit the softmax denominator as row 64
  scoresT[s, l] per (b, l-chunk), both heads in one 2-bank PSUM tile (one
  ScalarE exp covers both; no max subtraction: scores ~ N(0,1)), attn@v
  accumulated over s-tiles in PSUM, normalized via gpsimd partition_broadcast
  + DVE fast reciprocal.

Schedule: the attention inner loop is ScalarE-exp-paced (~1.0us per s-tile:
scores+av PE work is ~0.87us). All other PE work — batch-1 q/k/v projection
and all o-projections — is sliced into ~0.3-0.6us closures on a readiness-
gated queue and drip-fed one or two per s-tile period so the in-order PE
queue never blocks on unready work. PSUM is split into dedicated pools
(scores 2x2 banks, filler 1x2, attn@v 2) so a long-held filler tile can
never capture a slot the scores rotation needs. Each chunk's last `lag`
attn@v matmuls are carried into the next chunk's first periods, so the exp
pipeline drains under fresh scores instead of a boundary bubble. Startup
prioritizes k-tile-0 x/wq/wk DMAs and warms the exp table with a dummy
activation; the tail normalize broadcasts the reciprocal with an idle-PE
K=1 ones-matmul and stages output through the idle ScalarE.
"""

from collections import deque
from contextlib import ExitStack

import ml_dtypes
import numpy as np

import concourse.bacc as bacc
import concourse.mybir as mybir
import concourse.tile as tile
from concourse.bass_utils import run_bass_kernel_spmd

D_MODEL = 1024
N_HEAD = 16
HEAD_DIM = 64
B = 2
L = 2048
N_CORES = 8
HPC = N_HEAD // N_CORES  # heads per core
MLOC = HPC * HEAD_DIM  # 128: local d width per core

F32 = mybir.dt.float32
BF16 = mybir.dt.bfloat16
NPBF16 = ml_dtypes.bfloat16


def build_nc(Lb=L, lc_size=512, nch=512):
    """Build the per-core Bass program. Lb = sequence length per batch."""
    BLb = B * Lb
    KT = D_MODEL // 128  # 8 contraction tiles for the projections
    n_nch = BLb // nch  # projection column chunks
    st_per_nch = nch // 128  # s-tiles per projection chunk
    n_lc = Lb // lc_size  # attention l-chunks per batch
    n_st = Lb // 128  # s-tiles per batch
    upfront = Lb // nch  # batch-0 projection chunks emitted before attention

    nc = bacc.Bacc("TRN2", target_bir_lowering=False, debug=False)

    xT = nc.dram_tensor("xT", [D_MODEL, BLb], BF16, kind="ExternalInput").ap()
    wq = nc.dram_tensor("wq", [D_MODEL, MLOC], BF16, kind="ExternalInput").ap()
    wk = nc.dram_tensor("wk", [D_MODEL, MLOC], BF16, kind="ExternalInput").ap()
    wv = nc.dram_tensor("wv", [D_MODEL, MLOC], BF16, kind="ExternalInput").ap()
    wo = nc.dram_tensor("wo", [MLOC, D_MODEL], BF16, kind="ExternalInput").ap()
    bq = nc.dram_tensor("bq", [MLOC, 1], F32, kind="ExternalInput").ap()
    bk = nc.dram_tensor("bk", [MLOC, 1], F32, kind="ExternalInput").ap()
    bv = nc.dram_tensor("bv", [MLOC, 1], F32, kind="ExternalInput").ap()
    out = nc.dram_tensor("out", [BLb, D_MODEL], BF16, kind="ExternalOutput").ap()

    wqr = wq.rearrange("(k p) m -> p k m", p=128)
    wkr = wk.rearrange("(k p) m -> p k m", p=128)
    wvr = wv.rearrange("(k p) m -> p k m", p=128)

    with tile.TileContext(nc) as tc, ExitStack() as ctx:
        consts = ctx.enter_context(tc.tile_pool(name="consts", bufs=1))
        qk_sb = ctx.enter_context(tc.tile_pool(name="qk_sb", bufs=1))
        xt_pool = ctx.enter_context(tc.tile_pool(name="xt", bufs=2 * KT))
        # PSUM: scores get their own 2-slot rotation (2 banks each) so the
        # long-held projection/o-proj tile (its own 1-slot pool) can never
        # capture a slot the scores pipeline needs; av pool is a 2-bank
        # pair. Total 8 banks.
        big_ps = ctx.enter_context(tc.tile_pool(name="big_ps", bufs=2, space="PSUM"))
        fill_ps = ctx.enter_context(tc.tile_pool(name="fill_ps", bufs=1, space="PSUM"))
        av_ps = ctx.enter_context(tc.tile_pool(name="av_ps", bufs=1, space="PSUM"))
        exp_pool = ctx.enter_context(tc.tile_pool(name="expT", bufs=8))
        att_sb = ctx.enter_context(tc.tile_pool(name="att_sb", bufs=3))
        out_pool = ctx.enter_context(tc.tile_pool(name="out_sb", bufs=6))

        # ---- startup: priority DMA ordering -------------------------------
        # k-tile-0 x/wq/wk first on the sync queue so the first projection
        # matmul can fire as early as possible; the rest of the weights go on
        # the gpsimd queue (cheapest sequencer dispatch).
        wq_sb = consts.tile([128, KT, MLOC], BF16, tag="wq")
        wk_sb = consts.tile([128, KT, MLOC], BF16, tag="wk")
        wv_sb = consts.tile([128, KT, MLOC], BF16, tag="wv")
        wo_sb = consts.tile([128, D_MODEL], BF16, tag="wo")
        bq_sb = consts.tile([MLOC, 1], F32, tag="bq")
        bk_sb = consts.tile([MLOC, 1], F32, tag="bk")
        bv_sb = consts.tile([MLOC, 1], F32, tag="bv")

        loaded_xts = {}

        def load_xts_for(nc_i, eng):
            csl = slice(nc_i * nch, (nc_i + 1) * nch)
            xts = []
            for k in range(KT):
                xt = xt_pool.tile([128, nch], BF16, tag="xt", name="xt")
                eng.dma_start(xt[:], xT[128 * k : 128 * (k + 1), csl])
                xts.append(xt)
            loaded_xts[nc_i] = xts

        # chunk 0 x tiles + first q/k weight tiles, interleaved by priority
        xts0 = []
        for k in range(KT):
            xt = xt_pool.tile([128, nch], BF16, tag="xt", name="xt")
            xts0.append(xt)
        # Early dummy exp pulls ACT_TABLE_LOAD out of the attention phase
        # (emitted before the scalar-queue DMAs below).
        ones_bf = consts.tile([1, 128], BF16, tag="ones_bf")
        nc.vector.memset(ones_bf[:], 1.0)
        warm = consts.tile([1, 8], BF16, tag="actwarm")
        nc.scalar.activation(warm[:], ones_bf[0:1, 0:8],
                             mybir.ActivationFunctionType.Exp)

        nc.sync.dma_start(xts0[0][:], xT[0:128, 0:nch])
        nc.sync.dma_start(wq_sb[:, 0, :], wqr[:, 0, :])
        nc.sync.dma_start(wk_sb[:, 0, :], wkr[:, 0, :])
        nc.sync.dma_start(xts0[1][:], xT[128:256, 0:nch])
        nc.sync.dma_start(wq_sb[:, 1, :], wqr[:, 1, :])
        nc.sync.dma_start(wk_sb[:, 1, :], wkr[:, 1, :])
        nc.sync.dma_start(xts0[2][:], xT[256:384, 0:nch])
        # scalar queue is free until the first exp (~25us): bulk x tiles
        for k in range(3, KT):
            nc.scalar.dma_start(xts0[k][:], xT[128 * k : 128 * (k + 1), 0:nch])
        loaded_xts[0] = xts0
        if n_nch > 1:
            load_xts_for(1, nc.scalar)
        for k in range(2, KT):
            nc.gpsimd.dma_start(wq_sb[:, k, :], wqr[:, k, :])
            nc.gpsimd.dma_start(wk_sb[:, k, :], wkr[:, k, :])
        nc.gpsimd.dma_start(bq_sb[:], bq)
        nc.gpsimd.dma_start(bk_sb[:], bk)
        for k in range(KT):
            nc.gpsimd.dma_start(wv_sb[:, k, :], wvr[:, k, :])
        nc.gpsimd.dma_start(bv_sb[:], bv)
        nc.gpsimd.dma_start(wo_sb[:], wo)

        # Persistent activations.
        qT_sb = qk_sb.tile([128, BLb], BF16, tag="qT")  # [d_local, b*Lb+l]
        kT_sb = qk_sb.tile([128, BLb], BF16, tag="kT")
        # v (natural layout) + ones column: per (b, head): [128, n_st, 65]
        vaug = [
            [qk_sb.tile([128, n_st, HEAD_DIM + 1], BF16, tag=f"vaug{bi}{h}",
                        name=f"vaug{bi}{h}")
             for h in range(HPC)]
            for bi in range(B)
        ]
        for bi in range(B):
            for h in range(HPC):
                nc.vector.memset(vaug[bi][h][:, :, HEAD_DIM:], 1.0)

        # ---- projection chunk, as a list of small emission closures -------
        def proj_slices(nc_i, prefetch):
            """q/k/v projection for one 512-column chunk of x, split into
            KT + st_per_nch closures of ~0.3-0.5us of PE work each.
            `prefetch`: chunk id whose x tiles to DMA when this chunk's v
            phase starts (one chunk of lookahead)."""
            csl = slice(nc_i * nch, (nc_i + 1) * nch)
            cell = {}

            def qk_slice(k):
                def f():
                    if k == 0:
                        cell["ps"] = fill_ps.tile([128, 2, nch], F32, tag="fill",
                                                  name="ps_qk")
                    ps, xts = cell["ps"], loaded_xts[nc_i]
                    nc.tensor.matmul(ps[:, 0, :], wq_sb[:, k, :], xts[k][:],
                                     start=(k == 0), stop=(k == KT - 1))
                    nc.tensor.matmul(ps[:, 1, :], wk_sb[:, k, :], xts[k][:],
                                     start=(k == 0), stop=(k == KT - 1))
                    if k == KT - 1:
                        nc.vector.tensor_scalar_add(qT_sb[:, csl], ps[:, 0, :],
                                                    bq_sb[:])
                        nc.vector.tensor_scalar_add(kT_sb[:, csl], ps[:, 1, :],
                                                    bk_sb[:])
                return f

            def v_slice(stg):
                def f():
                    if stg == 0:
                        vpool = big_ps if nc_i < upfront else fill_ps
                        vtag = "big" if nc_i < upfront else "fill"
                        cell["psv"] = vpool.tile([128, nch], F32, tag=vtag,
                                                 name="ps_v")
                    ps_v, xts = cell["psv"], loaded_xts[nc_i]
                    ssl = slice(128 * stg, 128 * (stg + 1))
                    for k in range(KT):
                        nc.tensor.matmul(ps_v[:, ssl], xts[k][:, ssl],
                                         wv_sb[:, k, :],
                                         start=(k == 0), stop=(k == KT - 1))
                    if stg < st_per_nch - 1:
                        return
                    # Evacuations batched after ALL v matmuls: a per-group
                    # copy would WAR-serialize the next group's matmuls on
                    # the (tile-granular) ps_v readers through the DVE queue.
                    for g in range(st_per_nch):
                        st_g = nc_i * st_per_nch + g
                        bi, st_b = divmod(st_g, n_st)
                        for h in range(HPC):
                            nc.vector.tensor_copy(
                                vaug[bi][h][:, st_b, :HEAD_DIM],
                                ps_v[:, 128 * g + HEAD_DIM * h
                                     : 128 * g + HEAD_DIM * (h + 1)])
                    # All readers of this chunk's x tiles are emitted now;
                    # safe to prefetch chunk nc_i+2 into the slots this
                    # chunk's predecessor freed.
                    if prefetch is not None:
                        load_xts_for(prefetch, nc.sync)
                return f

            return ([qk_slice(k) for k in range(KT)]
                    + [v_slice(g) for g in range(st_per_nch)])

        def norm_den(avs_h, width):
            """Stage 1 of the normalize chain: denominator -> reciprocal,
            broadcast across partitions. Split from stage 2 so the DVE burst
            at a chunk boundary leaves gaps for filler-chunk evacuations."""
            den = att_sb.tile([1, 2, lc_size], F32, tag="den", name="den")
            rcp = att_sb.tile([128, 2, lc_size], F32, tag="rcp", name="rcp")
            bden = att_sb.tile([128, 2, lc_size], F32, tag="bden", name="bden")
            for h in range(HPC):
                nc.vector.tensor_copy(den[0:1, h, :width], avs_h[h][64:65, :width])
            nc.gpsimd.partition_broadcast(bden[:, :, :width], den[0:1, :, :width])
            nc.vector.reciprocal_approx_fast(rcp[:, :, :width],
                                             bden[:, :, :width])
            return rcp

        def norm_mul(avs_h, rcp, width):
            """Stage 2: scale by the reciprocal, add the v bias -> oT."""
            oT = att_sb.tile([128, lc_size], BF16, tag="oT", name="oT", bufs=6)
            for h in range(HPC):
                hsl = slice(64 * h, 64 * (h + 1))
                nc.vector.tensor_mul(oT[hsl, :width], avs_h[h][:HEAD_DIM, :width],
                                     rcp[:HEAD_DIM, h, :width])
                nc.vector.tensor_scalar_add(oT[hsl, :width], oT[hsl, :width],
                                            bv_sb[hsl, :])
            return oT[:, :width]

        def oproj_slice(oT, bi, loff, lt, tail=False):
            def f():
                # Tail o-projections rotate through the (idle) scores pool so
                # consecutive slices don't WAR-serialize on the single fill
                # slot, and stage out via the idle ScalarE half the time.
                pool, tag = (big_ps, "big") if tail else (fill_ps, "fill")
                ps_o = pool.tile([128, 2, 512], F32, tag=tag, name="ps_o")
                for dh in range(2):
                    nc.tensor.matmul(ps_o[:, dh, :],
                                     oT[:, 128 * lt : 128 * (lt + 1)],
                                     wo_sb[:, 512 * dh : 512 * (dh + 1)],
                                     start=True, stop=True)
                ob = out_pool.tile([128, D_MODEL], BF16, tag="ob")
                if tail and lt % 2 == 0:
                    nc.scalar.activation(ob[:],
                                         ps_o.rearrange("p a b -> p (a b)"),
                                         mybir.ActivationFunctionType.Copy)
                else:
                    nc.vector.tensor_copy(ob[:],
                                          ps_o.rearrange("p a b -> p (a b)"))
                nc.sync.dma_start(
                    out[bi * Lb + loff + 128 * lt
                        : bi * Lb + loff + 128 * (lt + 1), :], ob[:])
            return f

        # ---- upfront: batch-0 projections, emitted densely ----------------
        # x-tile prefetch chain: the preamble loaded chunks 0 and 1; each
        # chunk's last v slice loads chunk c+2 (fits the 2-chunk xt pool).
        def prefetch_of(nc_i):
            return nc_i + 2 if nc_i + 2 < n_nch else None

        for nc_i in range(upfront):
            for f in proj_slices(nc_i, prefetch_of(nc_i)):
                f()

        # ---- filler queue: batch-1 projections + o-projections ------------
        # Entries are (ready_period, closure): a closure is not popped before
        # its ready period, so o-proj slices never stall the PE on their
        # still-running DVE normalize chain.
        fillers = deque()
        period = [0]
        for nc_i in range(upfront, n_nch):
            for f in proj_slices(nc_i, prefetch_of(nc_i)):
                fillers.append((0, f))

        def pop_fillers(n, force=False):
            for _ in range(n):
                if not fillers:
                    return
                ready, f = fillers[0]
                if ready > period[0] and not force:
                    return
                fillers.popleft()
                f()

        # ---- attention chunks ---------------------------------------------
        chunks = []
        for bi in range(B):
            for lc in range(n_lc):
                chunks.append((bi, lc * lc_size, lc_size))

        prev = None  # previous chunk's state; its last avs are emitted here
        for ci, (bi, loff, width) in enumerate(chunks):
            lsl = slice(bi * Lb + loff, bi * Lb + loff + width)
            cellav = {}
            exs = [None] * n_st

            def do_sc(st):
                ssl = slice(bi * Lb + st * 128, bi * Lb + (st + 1) * 128)
                ps_sc = big_ps.tile([128, HPC, lc_size], F32, tag="big",
                                    name="ps_sc")
                for h in range(HPC):
                    hsl = slice(64 * h, 64 * (h + 1))
                    nc.tensor.matmul(ps_sc[:, h, :width], kT_sb[hsl, ssl],
                                     qT_sb[hsl, lsl],
                                     start=True, stop=True,
                                     tile_position=(64 * h, 0))
                ex = exp_pool.tile([128, HPC, lc_size], BF16, tag="ex",
                                   name="ex")
                nc.scalar.activation(ex[:, :, :width], ps_sc[:, :, :width],
                                     mybir.ActivationFunctionType.Exp,
                                     scale=1.0 / np.sqrt(HEAD_DIM))
                exs[st] = ex

            def do_av(st, bi=bi, width=width, exs=exs, cellav=cellav):
                for h in range(HPC):
                    nc.tensor.matmul(cellav["ps"][h][:], vaug[bi][h][:, st, :],
                                     exs[st][:, h, :width],
                                     start=(st == 0), stop=(st == n_st - 1))

            lag = 2 if n_st > 4 else 1
            # The previous chunk's last `lag` avs + its PSUM evacuation run
            # interleaved with this chunk's first scores: the exp pipeline
            # drains under fresh work instead of leaving a PE bubble at the
            # boundary.
            norm_pend = None
            for st in range(lag):
                do_sc(st)
                if prev is not None:
                    prev["do_av"](n_st - lag + st)
                    if st == lag - 1:
                        avs = att_sb.tile([HEAD_DIM + 1, 2, lc_size], F32,
                                          tag="avs", name="avs")
                        for h in range(HPC):
                            nc.vector.tensor_copy(avs[:, h, :prev["width"]],
                                                  prev["ps"][h][:])
                        norm_pend = ([avs[:, h, :] for h in range(HPC)],
                                     prev["bi"], prev["loff"], prev["width"])
                        prev = None
                pop_fillers(1)
                period[0] += 1
            cellav["ps"] = [av_ps.tile([HEAD_DIM + 1, lc_size], F32,
                                       tag=f"av{h}", name=f"av{h}")[:, :width]
                            for h in range(HPC)]
            if norm_pend is not None:
                rcp_p = norm_den(norm_pend[0], norm_pend[3])
            for st in range(lag, n_st):
                do_sc(st)
                if st == lag + 1 and norm_pend is not None:
                    oTp = norm_mul(norm_pend[0], rcp_p, norm_pend[3])
                    # Ready periods 14..17 relative to the chunk start: the
                    # o-proj slices land exactly in the boundary periods
                    # (last two of this chunk + the next chunk's lag periods)
                    # where the PE otherwise drains the 2-deep exp pipeline.
                    for lt in range(norm_pend[3] // 128):
                        fillers.append((period[0] + 11 + lt,
                                        oproj_slice(oTp, norm_pend[1],
                                                    norm_pend[2], lt)))
                    norm_pend = None
                pop_fillers(1)
                do_av(st - lag)
                period[0] += 1
            prev = {"do_av": do_av, "ps": cellav["ps"], "bi": bi,
                    "loff": loff, "width": width}

        # ---- tail: last chunk's normalize + o-proj ------------------------
        # The only normalize with nothing to hide under: broadcast via an
        # idle-PE K=1 matmul instead of gpsimd, and pipeline the scale +
        # o-proj per 256-l half so the PE starts while the DVE finishes.
        for st in range(n_st - lag, n_st):
            pop_fillers(2, force=True)
            prev["do_av"](st)
        pop_fillers(len(fillers), force=True)
        avs_h, bi_l, loff_l, width_l = (prev["ps"], prev["bi"], prev["loff"],
                                        prev["width"])
        den_t = att_sb.tile([1, 2, lc_size], BF16, tag="denb", name="denb")
        nc.vector.tensor_copy(den_t[0:1, 0, :width_l],
                              avs_h[0][64:65, :width_l])
        nc.scalar.activation(den_t[0:1, 1, :width_l],
                             avs_h[1][64:65, :width_l],
                             mybir.ActivationFunctionType.Copy)
        ps_r = fill_ps.tile([128, 2, lc_size], F32, tag="fill", name="ps_r")
        for h in range(HPC):
            nc.tensor.matmul(ps_r[:, h, :width_l], ones_bf[:],
                             den_t[0:1, h, :width_l], start=True, stop=True)
        rcp_t = att_sb.tile([128, 2, lc_size], F32, tag="rcp", name="rcp")
        nc.vector.reciprocal_approx_fast(rcp_t[:, :, :width_l],
                                         ps_r[:, :, :width_l])
        oT_t = att_sb.tile([128, lc_size], BF16, tag="oT", name="oT", bufs=6)
        half = width_l // 2
        for hw in range(2):
            wsl = slice(hw * half, (hw + 1) * half)
            for h in range(HPC):
                hsl = slice(64 * h, 64 * (h + 1))
                nc.vector.tensor_mul(oT_t[hsl, wsl], avs_h[h][:HEAD_DIM, wsl],
                                     rcp_t[:HEAD_DIM, h, wsl])
                nc.vector.tensor_scalar_add(oT_t[hsl, wsl], oT_t[hsl, wsl],
                                            bv_sb[hsl, :])
            for lt in range(hw * half // 128, (hw + 1) * half // 128):
                oproj_slice(oT_t[:, :width_l], bi_l, loff_l, lt, tail=True)()

    nc.compile()
    return nc


def make_in_maps(x, Wq, bq, Wk, bk, Wv, bv, Wo, Lb=L):
    """Per-core input dicts from full inputs."""
    BLb = B * Lb
    xT = np.ascontiguousarray(
        np.asarray(x, np.float32).reshape(BLb, D_MODEL).T).astype(NPBF16)
    Wq = np.asarray(Wq, np.float32).astype(NPBF16)
    Wk = np.asarray(Wk, np.float32).astype(NPBF16)
    Wv = np.asarray(Wv, np.float32).astype(NPBF16)
    Wo = np.asarray(Wo, np.float32).astype(NPBF16)
    in_maps = []
    for c in range(N_CORES):
        dsl = slice(MLOC * c, MLOC * (c + 1))
        in_maps.append({
            "xT": xT,
            "wq": np.ascontiguousarray(Wq[:, dsl]),
            "wk": np.ascontiguousarray(Wk[:, dsl]),
            "wv": np.ascontiguousarray(Wv[:, dsl]),
            "wo": np.ascontiguousarray(Wo[dsl, :]),
            "bq": np.ascontiguousarray(np.asarray(bq, np.float32)[dsl].reshape(MLOC, 1)),
            "bk": np.ascontiguousarray(np.asarray(bk, np.float32)[dsl].reshape(MLOC, 1)),
            "bv": np.ascontiguousarray(np.asarray(bv, np.float32)[dsl].reshape(MLOC, 1)),
        })
    return in_maps


_NC_CACHE = {}


def _get_nc():
    if "nc" not in _NC_CACHE:
        _NC_CACHE["nc"] = build_nc()
    return _NC_CACHE["nc"]


def kernel(x, Wq, bq, Wk, bk, Wv, bv, Wo, bo):
    nc = _get_nc()
    in_maps = make_in_maps(x, Wq, bq, Wk, bk, Wv, bv, Wo)
    res = run_bass_kernel_spmd(nc, in_maps, list(range(N_CORES)))
    acc = np.zeros((B * L, D_MODEL), dtype=np.float32)
    for c in range(N_CORES):
        acc += np.asarray(res.results[c]["out"], dtype=np.float32)
    acc += np.asarray(bo, dtype=np.float32)
    return acc.reshape(B, L, D_MODEL)



# revision 5
# speedup vs baseline: 1.0277x; 1.0277x over previous
"""Multihead attention (B=2, L=2048, D=1024, 16 heads) on 8 trn2 cores.

Sharding: tensor-parallel over heads — 2 heads per core. Each core computes
q/k/v projections for its 128 columns of Wq/Wk/Wv, full attention for its two
heads, and a partial output projection against its 128 rows of Wo. The host
sums the 8 bf16 partials and adds bo.

Compute is bf16 on the PE with fp32 PSUM accumulation. Layouts:
  qT/kT: [128(d_local), B*L]      — contraction-major for the scoresT matmuls
  vaug:  [s, 64]+ones column      — streamed rhs for the transposed-av; the
                                    ones column emits the softmax denominator
  scoresT[s, l] per (b, l-chunk), both heads in one 2-bank PSUM tile; exp on
  ScalarE (no max subtraction: scores ~ N(0,1)).

Transposed attn@v: out[l-tile, d] accumulates with exs (the exp'd scoresT
block) as the stationary weights and vaug as the 65-col stream — full PE
column utilization (vs M=65 in the d-major layout). The denominator lands
per-PARTITION (col 64 of each l-row), so normalization folds into a
diagonal-matmul transpose: D = I * rcp (DVE per-partition scale of a host-
provided identity), then oT[hd, l] = o_sb^T @ D on the PE — transpose and
per-head normalize in one matmul, no gpsimd broadcast. bv is added at the
oT evacuation (per-partition there), exact since sum(attn)=1 post-normalize.

PSUM (8 banks): scores 2x2, av 2x1 ([128,4,65] per head, 4 sub-bank matmul
accumulation regions), aux 2x1 (q/k/v projection accumulators run
sequentially one slot at a time, o-proj halves [128,512], oT transposes).

Schedule: ScalarE-exp-paced (~1.1us per s-tile; attention PE work is
~0.7us). Batch-1 projections drip-feed as group-atomic filler chains (a
projection chunk's accumulating PSUM tile must not have other aux
allocations interleaved); the per-chunk o-path (rcp/D/oT/o-proj) closures
are self-contained singles on a priority queue that runs between groups.
Each chunk's last `lag` attn@v groups carry into the next chunk's first
periods so the exp pipeline drains under fresh scores.
"""

from collections import deque
from contextlib import ExitStack

import ml_dtypes
import numpy as np

import concourse.bacc as bacc
import concourse.mybir as mybir
import concourse.tile as tile
from concourse.bass_utils import run_bass_kernel_spmd

D_MODEL = 1024
N_HEAD = 16
HEAD_DIM = 64
B = 2
L = 2048
N_CORES = 8
HPC = N_HEAD // N_CORES  # heads per core
MLOC = HPC * HEAD_DIM  # 128: local d width per core

F32 = mybir.dt.float32
BF16 = mybir.dt.bfloat16
NPBF16 = ml_dtypes.bfloat16


def build_nc(Lb=L, lc_size=512, nch=512, upfront=None):
    """Build the per-core Bass program. Lb = sequence length per batch."""
    BLb = B * Lb
    KT = D_MODEL // 128  # 8 contraction tiles for the projections
    n_nch = BLb // nch  # projection column chunks
    st_per_nch = nch // 128  # s-tiles per projection chunk
    n_lc = Lb // lc_size  # attention l-chunks per batch
    n_st = Lb // 128  # s-tiles per batch
    n_lt = lc_size // 128  # l-tiles (128 wide) per attention chunk
    if upfront is None:
        upfront = Lb // nch  # batch-0 proj chunks emitted before attention

    nc = bacc.Bacc("TRN2", target_bir_lowering=False, debug=False)

    xT = nc.dram_tensor("xT", [D_MODEL, BLb], BF16, kind="ExternalInput").ap()
    wq = nc.dram_tensor("wq", [D_MODEL, MLOC], BF16, kind="ExternalInput").ap()
    wk = nc.dram_tensor("wk", [D_MODEL, MLOC], BF16, kind="ExternalInput").ap()
    wv = nc.dram_tensor("wv", [D_MODEL, MLOC], BF16, kind="ExternalInput").ap()
    wo = nc.dram_tensor("wo", [MLOC, D_MODEL], BF16, kind="ExternalInput").ap()
    bq = nc.dram_tensor("bq", [MLOC, 1], F32, kind="ExternalInput").ap()
    bk = nc.dram_tensor("bk", [MLOC, 1], F32, kind="ExternalInput").ap()
    bv = nc.dram_tensor("bv", [MLOC, 1], F32, kind="ExternalInput").ap()
    ident = nc.dram_tensor("ident", [128, 128], BF16, kind="ExternalInput").ap()
    out = nc.dram_tensor("out", [BLb, D_MODEL], BF16, kind="ExternalOutput").ap()

    wqr = wq.rearrange("(k p) m -> p k m", p=128)
    wkr = wk.rearrange("(k p) m -> p k m", p=128)
    wvr = wv.rearrange("(k p) m -> p k m", p=128)

    with tile.TileContext(nc) as tc, ExitStack() as ctx:
        consts = ctx.enter_context(tc.tile_pool(name="consts", bufs=1))
        qk_sb = ctx.enter_context(tc.tile_pool(name="qk_sb", bufs=1))
        xt_pool = ctx.enter_context(tc.tile_pool(name="xt", bufs=2 * KT))
        # PSUM (8 banks): scores 2 slots x 2 banks; av 2 tags x 1 bank;
        # aux 2 slots x 1 bank.
        big_ps = ctx.enter_context(tc.tile_pool(name="big_ps", bufs=2, space="PSUM"))
        av_ps = ctx.enter_context(tc.tile_pool(name="av_ps", bufs=1, space="PSUM"))
        aux_ps = ctx.enter_context(tc.tile_pool(name="aux_ps", bufs=2, space="PSUM"))
        exp_pool = ctx.enter_context(tc.tile_pool(name="expT", bufs=6))
        att_sb = ctx.enter_context(tc.tile_pool(name="att_sb", bufs=2))
        d_pool = ctx.enter_context(tc.tile_pool(name="d_sb", bufs=HPC * 4))
        out_pool = ctx.enter_context(tc.tile_pool(name="out_sb", bufs=6))

        # ---- startup: priority DMA ordering -------------------------------
        wq_sb = consts.tile([128, KT, MLOC], BF16, tag="wq")
        wk_sb = consts.tile([128, KT, MLOC], BF16, tag="wk")
        wv_sb = consts.tile([128, KT, MLOC], BF16, tag="wv")
        wo_sb = consts.tile([128, D_MODEL], BF16, tag="wo")
        bq_sb = consts.tile([MLOC, 1], F32, tag="bq")
        bk_sb = consts.tile([MLOC, 1], F32, tag="bk")
        bv_sb = consts.tile([MLOC, 1], F32, tag="bv")
        id_sb = consts.tile([128, 128], BF16, tag="ident")

        loaded_xts = {}

        def load_xts_for(nc_i, eng):
            csl = slice(nc_i * nch, (nc_i + 1) * nch)
            xts = []
            for k in range(KT):
                xt = xt_pool.tile([128, nch], BF16, tag="xt", name="xt")
                eng.dma_start(xt[:], xT[128 * k : 128 * (k + 1), csl])
                xts.append(xt)
            loaded_xts[nc_i] = xts

        # chunk 0 x tiles + first q/k weight tiles, interleaved by priority
        xts0 = []
        for k in range(KT):
            xt = xt_pool.tile([128, nch], BF16, tag="xt", name="xt")
            xts0.append(xt)
        # Early dummy exp pulls ACT_TABLE_LOAD out of the attention phase.
        ones_bf = consts.tile([1, 128], BF16, tag="ones_bf")
        nc.vector.memset(ones_bf[:], 1.0)
        warm = consts.tile([1, 8], BF16, tag="actwarm")
        nc.scalar.activation(warm[:], ones_bf[0:1, 0:8],
                             mybir.ActivationFunctionType.Exp)

        nc.sync.dma_start(xts0[0][:], xT[0:128, 0:nch])
        nc.sync.dma_start(wq_sb[:, 0, :], wqr[:, 0, :])
        nc.sync.dma_start(wk_sb[:, 0, :], wkr[:, 0, :])
        nc.sync.dma_start(xts0[1][:], xT[128:256, 0:nch])
        nc.sync.dma_start(wq_sb[:, 1, :], wqr[:, 1, :])
        nc.sync.dma_start(wk_sb[:, 1, :], wkr[:, 1, :])
        nc.sync.dma_start(xts0[2][:], xT[256:384, 0:nch])
        # scalar queue is free until the first exp (~25us): bulk x tiles
        for k in range(3, KT):
            nc.scalar.dma_start(xts0[k][:], xT[128 * k : 128 * (k + 1), 0:nch])
        loaded_xts[0] = xts0
        if n_nch > 1:
            load_xts_for(1, nc.scalar)
        for k in range(2, KT):
            nc.gpsimd.dma_start(wq_sb[:, k, :], wqr[:, k, :])
            nc.gpsimd.dma_start(wk_sb[:, k, :], wkr[:, k, :])
        nc.gpsimd.dma_start(bq_sb[:], bq)
        nc.gpsimd.dma_start(bk_sb[:], bk)
        for k in range(KT):
            nc.gpsimd.dma_start(wv_sb[:, k, :], wvr[:, k, :])
        nc.gpsimd.dma_start(bv_sb[:], bv)
        nc.gpsimd.dma_start(wo_sb[:], wo)
        nc.gpsimd.dma_start(id_sb[:], ident)

        # Persistent activations.
        qT_sb = qk_sb.tile([128, BLb], BF16, tag="qT")  # [d_local, b*Lb+l]
        kT_sb = qk_sb.tile([128, BLb], BF16, tag="kT")
        # v (natural layout) + ones column: per (b, head): [128, n_st, 65]
        vaug = [
            [qk_sb.tile([128, n_st, HEAD_DIM + 1], BF16, tag=f"vaug{bi}{h}",
                        name=f"vaug{bi}{h}")
             for h in range(HPC)]
            for bi in range(B)
        ]
        for bi in range(B):
            for h in range(HPC):
                nc.vector.memset(vaug[bi][h][:, :, HEAD_DIM:], 1.0)

        # ---- projection chunk: group-atomic closure list ------------------
        # q, k, v accumulate sequentially, each holding a single 1-bank aux
        # slot; no other aux allocation may interleave within the group.
        def proj_slices(nc_i, prefetch):
            csl = slice(nc_i * nch, (nc_i + 1) * nch)
            cell = {}

            def pq_slice(j):
                def f():
                    if j == 0:
                        cell["ps"] = aux_ps.tile([128, nch], F32, tag="aux",
                                                 name="ps_q")
                    ps, xts = cell["ps"], loaded_xts[nc_i]
                    for k in (2 * j, 2 * j + 1):
                        nc.tensor.matmul(ps[:], wq_sb[:, k, :], xts[k][:],
                                         start=(k == 0), stop=(k == KT - 1))
                    if j == KT // 2 - 1:
                        nc.vector.tensor_scalar_add(qT_sb[:, csl], ps[:],
                                                    bq_sb[:])
                return f

            def pk_slice(j):
                def f():
                    if j == 0:
                        cell["ps"] = aux_ps.tile([128, nch], F32, tag="aux",
                                                 name="ps_k")
                    ps, xts = cell["ps"], loaded_xts[nc_i]
                    for k in (2 * j, 2 * j + 1):
                        nc.tensor.matmul(ps[:], wk_sb[:, k, :], xts[k][:],
                                         start=(k == 0), stop=(k == KT - 1))
                    if j == KT // 2 - 1:
                        nc.vector.tensor_scalar_add(kT_sb[:, csl], ps[:],
                                                    bk_sb[:])
                return f

            def v_slice(stg):
                def f():
                    if stg == 0:
                        cell["psv"] = aux_ps.tile([128, nch], F32, tag="aux",
                                                  name="ps_v")
                    ps_v, xts = cell["psv"], loaded_xts[nc_i]
                    ssl = slice(128 * stg, 128 * (stg + 1))
                    for k in range(KT):
                        nc.tensor.matmul(ps_v[:, ssl], xts[k][:, ssl],
                                         wv_sb[:, k, :],
                                         start=(k == 0), stop=(k == KT - 1))
                    if stg < st_per_nch - 1:
                        return
                    # Batched evacuation: one strided copy per head covers
                    # all s-groups of the chunk.
                    st0 = nc_i * st_per_nch
                    bi, st_b = divmod(st0, n_st)
                    psr = ps_v.rearrange("p (g c) -> p g c", g=st_per_nch)
                    for h in range(HPC):
                        nc.vector.tensor_copy(
                            vaug[bi][h][:, st_b : st_b + st_per_nch, :HEAD_DIM],
                            psr[:, :, HEAD_DIM * h : HEAD_DIM * (h + 1)])
                    if prefetch is not None:
                        load_xts_for(prefetch, nc.sync)
                return f

            return ([pq_slice(j) for j in range(KT // 2)]
                    + [pk_slice(j) for j in range(KT // 2)]
                    + [v_slice(g) for g in range(st_per_nch)])

        # ---- o-path for a finished chunk ----------------------------------
        # av psum [128 l, n_lt, 65] per head; col 64 = denominator per l.
        def opath_dmm(o_sb, rcp_sb, lt):
            """One l-tile: build D = I*rcp per head, normalize+transpose via
            diagonal matmul, evacuate oT with the bv bias add. Self-contained
            (one aux slot)."""
            def f():
                ds = []
                for h in range(HPC):
                    dt_ = d_pool.tile([128, 128], BF16, tag="D", name="D")
                    nc.vector.tensor_scalar_mul(dt_[:], id_sb[:],
                                                rcp_sb[:, h, lt : lt + 1])
                    ds.append(dt_)
                ps_oT = aux_ps.tile([128, 128], F32, tag="aux", name="ps_oT")
                for h in range(HPC):
                    nc.tensor.matmul(
                        ps_oT[HEAD_DIM * h : HEAD_DIM * (h + 1), :],
                        o_sb[:, lt, h, :HEAD_DIM], ds[h][:],
                        start=True, stop=True)
                oT = att_sb.tile([128, 128], BF16, tag="oT", name="oT",
                                 bufs=2 * n_lt)
                nc.vector.tensor_scalar_add(oT[:], ps_oT[:], bv_sb[:])
                return oT
            return f

        def oproj_slice(cell, lt, bi, loff):
            """Both halves of the o-projection for one l-tile."""
            def f():
                oT = cell[lt]
                r0 = bi * Lb + loff + 128 * lt
                for dh in range(2):
                    ps_o = aux_ps.tile([128, 512], F32, tag="aux", name="ps_o")
                    nc.tensor.matmul(ps_o[:], oT[:],
                                     wo_sb[:, 512 * dh : 512 * (dh + 1)],
                                     start=True, stop=True)
                    ob = out_pool.tile([128, 512], BF16, tag="ob")
                    if dh == 0:
                        nc.scalar.activation(ob[:], ps_o[:],
                                             mybir.ActivationFunctionType.Copy)
                    else:
                        nc.vector.tensor_copy(ob[:], ps_o[:])
                    nc.sync.dma_start(
                        out[r0 : r0 + 128, 512 * dh : 512 * (dh + 1)], ob[:])
            return f

        # ---- upfront: batch-0 projections, emitted densely ----------------
        def prefetch_of(nc_i):
            return nc_i + 2 if nc_i + 2 < n_nch else None

        for nc_i in range(upfront):
            for f in proj_slices(nc_i, prefetch_of(nc_i)):
                f()

        # ---- filler scheduling --------------------------------------------
        # proj_groups: group-atomic chains (one closure per period, no
        # interleaving once started). opath_q: ready-gated self-contained
        # singles with priority between groups.
        proj_groups = deque()
        period = [0]
        for nc_i in range(upfront, n_nch):
            proj_groups.append(deque(proj_slices(nc_i, prefetch_of(nc_i))))
        opath_q = deque()
        cur_group = [None]

        def pop_fillers(n, force=False):
            for _ in range(n):
                if cur_group[0]:
                    cur_group[0].popleft()()
                    if not cur_group[0]:
                        cur_group[0] = None
                elif opath_q and (opath_q[0][0] <= period[0] or force):
                    opath_q.popleft()[1]()
                elif proj_groups:
                    cur_group[0] = proj_groups.popleft()
                    cur_group[0].popleft()()
                    if not cur_group[0]:
                        cur_group[0] = None
                else:
                    return

        # ---- attention chunks ---------------------------------------------
        chunks = []
        for bi in range(B):
            for lc in range(n_lc):
                chunks.append((bi, lc * lc_size, lc_size))

        def emit_opath(prev, base_ready):
            """Evacuate the finished chunk's av psum, compute rcp, and queue
            the per-l-tile oT/o-proj closures."""
            o_sb = att_sb.tile([128, n_lt, HPC, HEAD_DIM + 1], BF16,
                               tag="o_sb", name="o_sb")
            rcp_sb = att_sb.tile([128, HPC, n_lt], F32, tag="rcp", name="rcp")
            for h in range(HPC):
                nc.vector.tensor_copy(o_sb[:, :, h, :], prev["ps"][h][:, :, :])
                nc.vector.reciprocal_approx_fast(
                    rcp_sb[:, h, :], prev["ps"][h][:, :, HEAD_DIM])
            cell = {}
            bi, loff = prev["bi"], prev["loff"]
            for lt in range(n_lt):
                dmm = opath_dmm(o_sb, rcp_sb, lt)

                def mk(lt=lt, dmm=dmm):
                    def g():
                        cell[lt] = dmm()
                    return g

                opath_q.append((base_ready + 2 * lt, mk()))
                opath_q.append((base_ready + 2 * lt + 1,
                                oproj_slice(cell, lt, bi, loff)))

        prev = None  # previous chunk's state; its last avs are emitted here
        for ci, (bi, loff, width) in enumerate(chunks):
            lsl = slice(bi * Lb + loff, bi * Lb + loff + width)
            cellav = {}
            exs = [None] * n_st

            def do_sc(st):
                ssl = slice(bi * Lb + st * 128, bi * Lb + (st + 1) * 128)
                ps_sc = big_ps.tile([128, HPC, lc_size], F32, tag="big",
                                    name="ps_sc")
                for h in range(HPC):
                    hsl = slice(64 * h, 64 * (h + 1))
                    nc.tensor.matmul(ps_sc[:, h, :width], kT_sb[hsl, ssl],
                                     qT_sb[hsl, lsl],
                                     start=True, stop=True,
                                     tile_position=(64 * h, 0))
                ex = exp_pool.tile([128, HPC, lc_size], BF16, tag="ex",
                                   name="ex")
                nc.scalar.activation(ex[:, :, :width], ps_sc[:, :, :width],
                                     mybir.ActivationFunctionType.Exp,
                                     scale=1.0 / np.sqrt(HEAD_DIM))
                exs[st] = ex

            def do_av(st, bi=bi, exs=exs, cellav=cellav):
                # One accumulation group per PSUM bank (per head): start
                # zero-marks the whole 2KB region, so the first touch of
                # every lt sub-region overwrites; only (st=0, lt=0) starts
                # and only the final write stops.
                for h in range(HPC):
                    for lt in range(n_lt):
                        nc.tensor.matmul(
                            cellav["ps"][h][:, lt, :],
                            exs[st][:, h, 128 * lt : 128 * (lt + 1)],
                            vaug[bi][h][:, st, :],
                            start=(st == 0 and lt == 0),
                            stop=(st == n_st - 1 and lt == n_lt - 1))

            lag = 2 if n_st > 4 else 1
            # Previous chunk's last `lag` avs + its o-path run interleaved
            # with this chunk's first scores.
            for st in range(lag):
                do_sc(st)
                if prev is not None:
                    prev["do_av"](n_st - lag + st)
                    if st == lag - 1:
                        emit_opath(prev, period[0] + 2)
                        prev = None
                pop_fillers(1)
                period[0] += 1
            cellav["ps"] = [av_ps.tile([128, n_lt, HEAD_DIM + 1], F32,
                                       tag=f"av{h}", name=f"av{h}")
                            for h in range(HPC)]
            for st in range(lag, n_st):
                do_sc(st)
                pop_fillers(1)
                do_av(st - lag)
                period[0] += 1
            prev = {"do_av": do_av, "ps": cellav["ps"], "bi": bi,
                    "loff": loff, "width": width}

        # ---- tail: last chunk's o-path -------------------------------------
        for st in range(n_st - lag, n_st):
            pop_fillers(2, force=True)
            prev["do_av"](st)
        emit_opath(prev, period[0])
        while cur_group[0] or opath_q or proj_groups:
            pop_fillers(1, force=True)

    nc.compile()
    return nc


def make_in_maps(x, Wq, bq, Wk, bk, Wv, bv, Wo, Lb=L):
    """Per-core input dicts from full inputs."""
    BLb = B * Lb
    xT = np.ascontiguousarray(
        np.asarray(x, np.float32).reshape(BLb, D_MODEL).T).astype(NPBF16)
    Wq = np.asarray(Wq, np.float32).astype(NPBF16)
    Wk = np.asarray(Wk, np.float32).astype(NPBF16)
    Wv = np.asarray(Wv, np.float32).astype(NPBF16)
    Wo = np.asarray(Wo, np.float32).astype(NPBF16)
    ident = np.eye(128, dtype=NPBF16)
    in_maps = []
    for c in range(N_CORES):
        dsl = slice(MLOC * c, MLOC * (c + 1))
        in_maps.append({
            "xT": xT,
            "wq": np.ascontiguousarray(Wq[:, dsl]),
            "wk": np.ascontiguousarray(Wk[:, dsl]),
            "wv": np.ascontiguousarray(Wv[:, dsl]),
            "wo": np.ascontiguousarray(Wo[dsl, :]),
            "bq": np.ascontiguousarray(np.asarray(bq, np.float32)[dsl].reshape(MLOC, 1)),
            "bk": np.ascontiguousarray(np.asarray(bk, np.float32)[dsl].reshape(MLOC, 1)),
            "bv": np.ascontiguousarray(np.asarray(bv, np.float32)[dsl].reshape(MLOC, 1)),
            "ident": ident,
        })
    return in_maps


_NC_CACHE = {}


def _get_nc():
    if "nc" not in _NC_CACHE:
        _NC_CACHE["nc"] = build_nc()
    return _NC_CACHE["nc"]


def kernel(x, Wq, bq, Wk, bk, Wv, bv, Wo, bo):
    nc = _get_nc()
    in_maps = make_in_maps(x, Wq, bq, Wk, bk, Wv, bv, Wo)
    res = run_bass_kernel_spmd(nc, in_maps, list(range(N_CORES)))
    acc = np.zeros((B * L, D_MODEL), dtype=np.float32)
    for c in range(N_CORES):
        acc += np.asarray(res.results[c]["out"], dtype=np.float32)
    acc += np.asarray(bo, dtype=np.float32)
    return acc.reshape(B, L, D_MODEL)


# revision 10
# speedup vs baseline: 1.0710x; 1.0422x over previous
"""Multihead attention (B=2, L=2048, D=1024, 16 heads) on 8 trn2 cores.

Sharding: tensor-parallel over heads — 2 heads per core. Each core computes
q/k/v projections for its 128 columns of Wq/Wk/Wv, full attention for its two
heads, and a partial output projection against its 128 rows of Wo. The host
sums the 8 bf16 partials and adds bo.

Compute is bf16 on the PE with fp32 PSUM accumulation. Layouts:
  qT/kT: [128(d_local), B*L]      — contraction-major for the scoresT matmuls
  vaug:  [s, 64]+ones column      — streamed rhs for the transposed-av; the
                                    ones column emits the softmax denominator
  scoresT[s, l] per (b, l-chunk), both heads in one 2-bank PSUM tile; exp on
  ScalarE (no max subtraction: scores ~ N(0,1)).

Transposed attn@v: out[l-tile, d] accumulates with exs (the exp'd scoresT
block) as the stationary weights and vaug as the 65-col stream — full PE
column utilization (vs M=65 in the d-major layout). The denominator lands
per-PARTITION (col 64 of each l-row), so normalization folds into a
diagonal-matmul transpose: D = I * rcp (DVE per-partition scale of a host-
provided identity), then oT[hd, l] = o_sb^T @ D on the PE — transpose and
per-head normalize in one matmul, no gpsimd broadcast. bv is added at the
oT evacuation (per-partition there), exact since sum(attn)=1 post-normalize.

PSUM (8 banks): scores 2x2, av 2x1 ([128,4,65] per head, 4 sub-bank matmul
accumulation regions), aux 2x1 (q/k/v projection accumulators run
sequentially one slot at a time, o-proj halves [128,512], oT transposes).

Schedule: ScalarE-exp-paced (~1.1us per s-tile; attention PE work is
~0.7us). Batch-1 projections drip-feed as group-atomic filler chains (a
projection chunk's accumulating PSUM tile must not have other aux
allocations interleaved); the per-chunk o-path (rcp/D/oT/o-proj) closures
are self-contained singles on a priority queue that runs between groups.
Each chunk's last `lag` attn@v groups carry into the next chunk's first
periods so the exp pipeline drains under fresh scores.
"""

from collections import deque
from contextlib import ExitStack

import ml_dtypes
import numpy as np

import concourse.bacc as bacc
import concourse.mybir as mybir
import concourse.tile as tile
from concourse.bass_utils import run_bass_kernel_spmd

D_MODEL = 1024
N_HEAD = 16
HEAD_DIM = 64
B = 2
L = 2048
N_CORES = 8
HPC = N_HEAD // N_CORES  # heads per core
MLOC = HPC * HEAD_DIM  # 128: local d width per core

F32 = mybir.dt.float32
BF16 = mybir.dt.bfloat16
NPBF16 = ml_dtypes.bfloat16


def build_nc(Lb=L, lc_size=512, nch=512, upfront=None):
    """Build the per-core Bass program. Lb = sequence length per batch."""
    BLb = B * Lb
    KT = D_MODEL // 128  # 8 contraction tiles for the projections
    n_nch = BLb // nch  # projection column chunks
    st_per_nch = nch // 128  # s-tiles per projection chunk
    n_lc = Lb // lc_size  # attention l-chunks per batch
    n_st = Lb // 128  # s-tiles per batch
    n_lt = lc_size // 128  # l-tiles (128 wide) per attention chunk
    if upfront is None:
        upfront = Lb // nch  # batch-0 proj chunks emitted before attention

    nc = bacc.Bacc("TRN2", target_bir_lowering=False, debug=False)

    xT = nc.dram_tensor("xT", [D_MODEL, BLb], BF16, kind="ExternalInput").ap()
    wq = nc.dram_tensor("wq", [D_MODEL, MLOC], BF16, kind="ExternalInput").ap()
    wk = nc.dram_tensor("wk", [D_MODEL, MLOC], BF16, kind="ExternalInput").ap()
    wv = nc.dram_tensor("wv", [D_MODEL, MLOC], BF16, kind="ExternalInput").ap()
    wo = nc.dram_tensor("wo", [MLOC, D_MODEL], BF16, kind="ExternalInput").ap()
    bq = nc.dram_tensor("bq", [MLOC, 1], F32, kind="ExternalInput").ap()
    bk = nc.dram_tensor("bk", [MLOC, 1], F32, kind="ExternalInput").ap()
    bv = nc.dram_tensor("bv", [MLOC, 1], F32, kind="ExternalInput").ap()
    ident = nc.dram_tensor("ident", [128, 128], BF16, kind="ExternalInput").ap()
    out = nc.dram_tensor("out", [BLb, D_MODEL], BF16, kind="ExternalOutput").ap()

    wqr = wq.rearrange("(k p) m -> p k m", p=128)
    wkr = wk.rearrange("(k p) m -> p k m", p=128)
    wvr = wv.rearrange("(k p) m -> p k m", p=128)

    with tile.TileContext(nc) as tc, ExitStack() as ctx:
        consts = ctx.enter_context(tc.tile_pool(name="consts", bufs=1))
        qk_sb = ctx.enter_context(tc.tile_pool(name="qk_sb", bufs=1))
        xt_pool = ctx.enter_context(tc.tile_pool(name="xt", bufs=2 * KT))
        # PSUM (8 banks): scores 2 slots x 2 banks; av 2 tags x 1 bank;
        # aux 2 slots x 1 bank.
        big_ps = ctx.enter_context(tc.tile_pool(name="big_ps", bufs=2, space="PSUM"))
        av_ps = ctx.enter_context(tc.tile_pool(name="av_ps", bufs=1, space="PSUM"))
        aux_ps = ctx.enter_context(tc.tile_pool(name="aux_ps", bufs=2, space="PSUM"))
        exp_pool = ctx.enter_context(tc.tile_pool(name="expT", bufs=6))
        att_sb = ctx.enter_context(tc.tile_pool(name="att_sb", bufs=2))
        d_pool = ctx.enter_context(tc.tile_pool(name="d_sb", bufs=HPC * 4))
        out_pool = ctx.enter_context(tc.tile_pool(name="out_sb", bufs=6))

        # ---- startup: priority DMA ordering -------------------------------
        wq_sb = consts.tile([128, KT, MLOC], BF16, tag="wq")
        wk_sb = consts.tile([128, KT, MLOC], BF16, tag="wk")
        wv_sb = consts.tile([128, KT, MLOC], BF16, tag="wv")
        wo_sb = consts.tile([128, D_MODEL], BF16, tag="wo")
        bq_sb = consts.tile([MLOC, 1], F32, tag="bq")
        bk_sb = consts.tile([MLOC, 1], F32, tag="bk")
        bv_sb = consts.tile([MLOC, 1], F32, tag="bv")
        id_sb = consts.tile([128, 128], BF16, tag="ident")

        loaded_xts = {}

        def load_xts_for(nc_i, eng):
            csl = slice(nc_i * nch, (nc_i + 1) * nch)
            xts = []
            for k in range(KT):
                xt = xt_pool.tile([128, nch], BF16, tag="xt", name="xt")
                eng.dma_start(xt[:], xT[128 * k : 128 * (k + 1), csl])
                xts.append(xt)
            loaded_xts[nc_i] = xts

        # chunk 0 x tiles + first q/k weight tiles, interleaved by priority
        xts0 = []
        for k in range(KT):
            xt = xt_pool.tile([128, nch], BF16, tag="xt", name="xt")
            xts0.append(xt)
        # Early dummy exp pulls ACT_TABLE_LOAD out of the attention phase.
        ones_bf = consts.tile([1, 128], BF16, tag="ones_bf")
        nc.vector.memset(ones_bf[:], 1.0)
        warm = consts.tile([1, 8], BF16, tag="actwarm")
        nc.scalar.activation(warm[:], ones_bf[0:1, 0:8],
                             mybir.ActivationFunctionType.Exp)

        nc.sync.dma_start(xts0[0][:], xT[0:128, 0:nch])
        nc.sync.dma_start(wq_sb[:, 0, :], wqr[:, 0, :])
        nc.sync.dma_start(wk_sb[:, 0, :], wkr[:, 0, :])
        nc.sync.dma_start(xts0[1][:], xT[128:256, 0:nch])
        nc.sync.dma_start(wq_sb[:, 1, :], wqr[:, 1, :])
        nc.sync.dma_start(wk_sb[:, 1, :], wkr[:, 1, :])
        nc.sync.dma_start(xts0[2][:], xT[256:384, 0:nch])
        # scalar queue is free until the first exp (~25us): bulk x tiles
        for k in range(3, KT):
            nc.scalar.dma_start(xts0[k][:], xT[128 * k : 128 * (k + 1), 0:nch])
        loaded_xts[0] = xts0
        if n_nch > 1:
            load_xts_for(1, nc.scalar)
        for k in range(2, KT):
            nc.gpsimd.dma_start(wq_sb[:, k, :], wqr[:, k, :])
            nc.gpsimd.dma_start(wk_sb[:, k, :], wkr[:, k, :])
        nc.gpsimd.dma_start(bq_sb[:], bq)
        nc.gpsimd.dma_start(bk_sb[:], bk)
        for k in range(KT):
            nc.gpsimd.dma_start(wv_sb[:, k, :], wvr[:, k, :])
        nc.gpsimd.dma_start(bv_sb[:], bv)
        nc.gpsimd.dma_start(wo_sb[:], wo)
        nc.gpsimd.dma_start(id_sb[:], ident)

        # Persistent activations.
        qT_sb = qk_sb.tile([128, BLb], BF16, tag="qT")  # [d_local, b*Lb+l]
        kT_sb = qk_sb.tile([128, BLb], BF16, tag="kT")
        # v (natural layout) + ones column: per (b, head): [128, n_st, 65]
        vaug = [
            [qk_sb.tile([128, n_st, HEAD_DIM + 1], BF16, tag=f"vaug{bi}{h}",
                        name=f"vaug{bi}{h}")
             for h in range(HPC)]
            for bi in range(B)
        ]
        for bi in range(B):
            for h in range(HPC):
                nc.vector.memset(vaug[bi][h][:, :, HEAD_DIM:], 1.0)

        # ---- projection chunk: group-atomic closure list ------------------
        # q, k, v accumulate sequentially, each holding a single 1-bank aux
        # slot; no other aux allocation may interleave within the group.
        def proj_slices(nc_i, prefetch):
            csl = slice(nc_i * nch, (nc_i + 1) * nch)
            cell = {}

            def pq_slice(j):
                def f():
                    if j == 0:
                        cell["ps"] = aux_ps.tile([128, nch], F32, tag="aux",
                                                 name="ps_q")
                    ps, xts = cell["ps"], loaded_xts[nc_i]
                    for k in (2 * j, 2 * j + 1):
                        nc.tensor.matmul(ps[:], wq_sb[:, k, :], xts[k][:],
                                         start=(k == 0), stop=(k == KT - 1))
                    if j == KT // 2 - 1:
                        nc.vector.tensor_scalar_add(qT_sb[:, csl], ps[:],
                                                    bq_sb[:])
                return f

            def pk_slice(j):
                def f():
                    if j == 0:
                        cell["ps"] = aux_ps.tile([128, nch], F32, tag="aux",
                                                 name="ps_k")
                    ps, xts = cell["ps"], loaded_xts[nc_i]
                    for k in (2 * j, 2 * j + 1):
                        nc.tensor.matmul(ps[:], wk_sb[:, k, :], xts[k][:],
                                         start=(k == 0), stop=(k == KT - 1))
                    if j == KT // 2 - 1:
                        nc.vector.tensor_scalar_add(kT_sb[:, csl], ps[:],
                                                    bk_sb[:])
                return f

            def v_slice(stg):
                def f():
                    if stg == 0:
                        cell["psv"] = aux_ps.tile([128, nch], F32, tag="aux",
                                                  name="ps_v")
                    ps_v, xts = cell["psv"], loaded_xts[nc_i]
                    ssl = slice(128 * stg, 128 * (stg + 1))
                    for k in range(KT):
                        nc.tensor.matmul(ps_v[:, ssl], xts[k][:, ssl],
                                         wv_sb[:, k, :],
                                         start=(k == 0), stop=(k == KT - 1))
                    if stg < st_per_nch - 1:
                        return
                    # Batched evacuation: one strided copy per head covers
                    # all s-groups of the chunk.
                    st0 = nc_i * st_per_nch
                    bi, st_b = divmod(st0, n_st)
                    psr = ps_v.rearrange("p (g c) -> p g c", g=st_per_nch)
                    for h in range(HPC):
                        nc.vector.tensor_copy(
                            vaug[bi][h][:, st_b : st_b + st_per_nch, :HEAD_DIM],
                            psr[:, :, HEAD_DIM * h : HEAD_DIM * (h + 1)])
                    if prefetch is not None:
                        load_xts_for(prefetch, nc.sync)
                return f

            return ([pq_slice(j) for j in range(KT // 2)]
                    + [pk_slice(j) for j in range(KT // 2)]
                    + [v_slice(g) for g in range(st_per_nch)])

        # ---- o-path for a finished chunk ----------------------------------
        # av psum [128 l, n_lt, 65] per head; col 64 = denominator per l.
        def opath_dmm_all(o_sb, rcp_sb, cell):
            """All l-tiles' normalize+transpose diagonal matmuls in one
            closure: one column-tiling mode episode for the whole chunk.
            The two heads' D-matmuls run concurrently on column tiles.
            Uses both aux slots (self-contained: allocate, matmul, evac)."""
            def f():
                ds = [[None] * n_lt for _ in range(HPC)]
                for lt in range(n_lt):
                    for h in range(HPC):
                        dt_ = d_pool.tile([128, 128], BF16, tag="D", name="D")
                        nc.vector.tensor_scalar_mul(dt_[:], id_sb[:],
                                                    rcp_sb[:, h, lt : lt + 1])
                        ds[h][lt] = dt_
                for half in range(n_lt // 2):
                    ps_oT = aux_ps.tile([128, 2, 128], F32, tag="aux",
                                        name="ps_oT")
                    for j in range(2):
                        lt = 2 * half + j
                        for h in range(HPC):
                            nc.tensor.matmul(
                                ps_oT[HEAD_DIM * h : HEAD_DIM * (h + 1), j, :],
                                o_sb[:, lt, h, :HEAD_DIM], ds[h][lt][:],
                                start=True, stop=True)
                    for j in range(2):
                        oT = att_sb.tile([128, 128], BF16, tag="oT", name="oT",
                                         bufs=2 * n_lt)
                        nc.vector.tensor_scalar_add(oT[:], ps_oT[:, j, :],
                                                    bv_sb[:])
                        cell[2 * half + j] = oT
            return f

        def oproj_slice(cell, lt, bi, loff):
            """Both halves of the o-projection for one l-tile."""
            def f():
                oT = cell[lt]
                r0 = bi * Lb + loff + 128 * lt
                for dh in range(2):
                    ps_o = aux_ps.tile([128, 512], F32, tag="aux", name="ps_o")
                    nc.tensor.matmul(ps_o[:], oT[:],
                                     wo_sb[:, 512 * dh : 512 * (dh + 1)],
                                     start=True, stop=True)
                    ob = out_pool.tile([128, 512], BF16, tag="ob")
                    nc.vector.tensor_copy(ob[:], ps_o[:])
                    nc.sync.dma_start(
                        out[r0 : r0 + 128, 512 * dh : 512 * (dh + 1)], ob[:])
            return f

        # ---- upfront: batch-0 projections, emitted densely ----------------
        def prefetch_of(nc_i):
            return nc_i + 2 if nc_i + 2 < n_nch else None

        for nc_i in range(upfront):
            for f in proj_slices(nc_i, prefetch_of(nc_i)):
                f()

        # ---- filler scheduling --------------------------------------------
        # proj_groups: group-atomic chains (one closure per period, no
        # interleaving once started). opath_q: ready-gated self-contained
        # singles with priority between groups.
        proj_groups = deque()
        period = [0]
        for nc_i in range(upfront, n_nch):
            proj_groups.append(deque(proj_slices(nc_i, prefetch_of(nc_i))))
        opath_q = deque()
        cur_group = [None]

        def pop_fillers(n, force=False):
            for _ in range(n):
                if cur_group[0]:
                    cur_group[0].popleft()()
                    if not cur_group[0]:
                        cur_group[0] = None
                elif opath_q and (opath_q[0][0] <= period[0] or force):
                    opath_q.popleft()[1]()
                elif proj_groups:
                    cur_group[0] = proj_groups.popleft()
                    cur_group[0].popleft()()
                    if not cur_group[0]:
                        cur_group[0] = None
                else:
                    return

        # ---- attention chunks ---------------------------------------------
        chunks = []
        for bi in range(B):
            for lc in range(n_lc):
                chunks.append((bi, lc * lc_size, lc_size))

        def emit_opath(prev, base_ready):
            """Evacuate the finished chunk's av psum, compute rcp, and queue
            the per-l-tile oT/o-proj closures."""
            o_sb = att_sb.tile([128, n_lt, HPC, HEAD_DIM + 1], BF16,
                               tag="o_sb", name="o_sb")
            rcp_sb = att_sb.tile([128, HPC, n_lt], F32, tag="rcp", name="rcp")
            for h in range(HPC):
                nc.vector.tensor_copy(o_sb[:, :, h, :], prev["ps"][h][:, :, :])
                nc.vector.reciprocal_approx_fast(
                    rcp_sb[:, h, :], prev["ps"][h][:, :, HEAD_DIM])
            cell = {}
            bi, loff = prev["bi"], prev["loff"]
            opath_q.append((base_ready, opath_dmm_all(o_sb, rcp_sb, cell)))
            for lt in range(n_lt):
                opath_q.append((base_ready + 2 + lt,
                                oproj_slice(cell, lt, bi, loff)))

        prev = None  # previous chunk's state; its last avs are emitted here
        for ci, (bi, loff, width) in enumerate(chunks):
            lsl = slice(bi * Lb + loff, bi * Lb + loff + width)
            cellav = {}
            exs = [None] * n_st

            def do_sc(st):
                ssl = slice(bi * Lb + st * 128, bi * Lb + (st + 1) * 128)
                ps_sc = big_ps.tile([128, HPC, lc_size], F32, tag="big",
                                    name="ps_sc")
                for h in range(HPC):
                    hsl = slice(64 * h, 64 * (h + 1))
                    nc.tensor.matmul(ps_sc[:, h, :width], kT_sb[hsl, ssl],
                                     qT_sb[hsl, lsl],
                                     start=True, stop=True,
                                     tile_position=(64 * h, 0))
                ex = exp_pool.tile([128, HPC, lc_size], BF16, tag="ex",
                                   name="ex")
                nc.scalar.activation(ex[:, :, :width], ps_sc[:, :, :width],
                                     mybir.ActivationFunctionType.Exp,
                                     scale=1.0 / np.sqrt(HEAD_DIM))
                exs[st] = ex

            def do_av(st, bi=bi, exs=exs, cellav=cellav):
                # One accumulation group per PSUM bank (per head): start
                # zero-marks the whole 2KB region, so the first touch of
                # every lt sub-region overwrites; only (st=0, lt=0) starts
                # and only the final write stops.
                for h in range(HPC):
                    for lt in range(n_lt):
                        nc.tensor.matmul(
                            cellav["ps"][h][:, lt, :],
                            exs[st][:, h, 128 * lt : 128 * (lt + 1)],
                            vaug[bi][h][:, st, :],
                            start=(st == 0 and lt == 0),
                            stop=(st == n_st - 1 and lt == n_lt - 1))

            lag = 2 if n_st > 4 else 1
            # Previous chunk's last `lag` avs + its o-path run interleaved
            # with this chunk's first scores.
            for st in range(lag):
                do_sc(st)
                if prev is not None:
                    prev["do_av"](n_st - lag + st)
                    if st == lag - 1:
                        emit_opath(prev, period[0] + 2)
                        prev = None
                pop_fillers(1)
                period[0] += 1
            cellav["ps"] = [av_ps.tile([128, n_lt, HEAD_DIM + 1], F32,
                                       tag=f"av{h}", name=f"av{h}")
                            for h in range(HPC)]
            for st in range(lag, n_st):
                do_sc(st)
                do_av(st - lag)
                pop_fillers(1)
                period[0] += 1
            prev = {"do_av": do_av, "ps": cellav["ps"], "bi": bi,
                    "loff": loff, "width": width}

        # ---- tail: last chunk's o-path -------------------------------------
        for st in range(n_st - lag, n_st):
            pop_fillers(2, force=True)
            prev["do_av"](st)
        emit_opath(prev, period[0])
        while cur_group[0] or opath_q or proj_groups:
            pop_fillers(1, force=True)

    nc.compile()
    return nc


def make_in_maps(x, Wq, bq, Wk, bk, Wv, bv, Wo, Lb=L):
    """Per-core input dicts from full inputs."""
    BLb = B * Lb
    xT = np.ascontiguousarray(
        np.asarray(x, np.float32).reshape(BLb, D_MODEL).T).astype(NPBF16)
    Wq = np.asarray(Wq, np.float32).astype(NPBF16)
    Wk = np.asarray(Wk, np.float32).astype(NPBF16)
    Wv = np.asarray(Wv, np.float32).astype(NPBF16)
    Wo = np.asarray(Wo, np.float32).astype(NPBF16)
    ident = np.eye(128, dtype=NPBF16)
    in_maps = []
    for c in range(N_CORES):
        dsl = slice(MLOC * c, MLOC * (c + 1))
        in_maps.append({
            "xT": xT,
            "wq": np.ascontiguousarray(Wq[:, dsl]),
            "wk": np.ascontiguousarray(Wk[:, dsl]),
            "wv": np.ascontiguousarray(Wv[:, dsl]),
            "wo": np.ascontiguousarray(Wo[dsl, :]),
            "bq": np.ascontiguousarray(np.asarray(bq, np.float32)[dsl].reshape(MLOC, 1)),
            "bk": np.ascontiguousarray(np.asarray(bk, np.float32)[dsl].reshape(MLOC, 1)),
            "bv": np.ascontiguousarray(np.asarray(bv, np.float32)[dsl].reshape(MLOC, 1)),
            "ident": ident,
        })
    return in_maps


_NC_CACHE = {}


def _get_nc():
    if "nc" not in _NC_CACHE:
        _NC_CACHE["nc"] = build_nc()
    return _NC_CACHE["nc"]


def kernel(x, Wq, bq, Wk, bk, Wv, bv, Wo, bo):
    nc = _get_nc()
    in_maps = make_in_maps(x, Wq, bq, Wk, bk, Wv, bv, Wo)
    res = run_bass_kernel_spmd(nc, in_maps, list(range(N_CORES)))
    acc = np.zeros((B * L, D_MODEL), dtype=np.float32)
    for c in range(N_CORES):
        acc += np.asarray(res.results[c]["out"], dtype=np.float32)
    acc += np.asarray(bo, dtype=np.float32)
    return acc.reshape(B, L, D_MODEL)
